# revision 1
# baseline (speedup 1.0000x reference)
"""Trainium2 Bass kernel for the DMP (dynamic movement primitives) rollout.

Math: the reference rollout is, per dimension d, a linear 2-state recurrence
    s_t = A s_{t-1} + B u_t,   s = [y; dy],  s_0 = [y0; 0]
with constant A (2x2), B = [dt^2; dt], and forcing
    u_t[d] = ALPHA_Y*BETA_Y*g[d] + sum_j phi_t[j] * weights[d,j]*(g[d]-y0[d])
where phi_t[j] = WEIGHT_SCALE * psi_t[j] * x_t / sum(psi_t) depends only on
constants (x_t = decay^t is input-independent).  By superposition the whole
trajectory factors through an input-independent basis:
    y_t[d], dy_t[d] = sum_m BB[t, comp, m] * coeff[m, d]       (m = 0..26)
with channels m = 0..24 the 25 basis-forced responses (coeff w[:,j]*(g-y0)),
m = 25 the homogeneous response (coeff y0), m = 26 the step response with
ALPHA_Y*BETA_Y folded in (coeff g).

Per core (time rows sharded across 8 cores, no cross-core comm):
  - coeff (27 x 1024) is computed on device from the raw y0/g/weights inputs
    (DVE stream transposes + a DMA partition-broadcast of g-y0),
  - the y/dy output blocks are a [2502, 27] @ [27, 1024] tensor-engine matmul,
  - the y0-replica block is written by broadcast DMA from an SBUF staging
    tile (no HBM reads, no compute).
"""

import numpy as np

DIM = 1024
NB = 25
ALPHA_X = 1.0
DT = 0.001
MAX_TIME = 10.0
TAU = 1.0
ALPHA_Y = 25.0
BETA_Y = 6.25
WEIGHT_SCALE = 1000.0
T = int(MAX_TIME / DT) + 1        # 10001

NCORES = 8
RPC = 1251                        # t-rows per core; 8*1251 = 10008 >= T
R2 = RPC * 2                      # 2502 matmul rows per core (y and dy)
R2PAD = 2560                      # 20 tiles of 128
NMT = R2PAD // 128                # 20
M = 2 + NB                        # 27 basis channels

# tensor-engine precision mode for the main matmul:
#   "f32"   exact fp32 (4 cyc/row)
#   "f32r"  hw fast-fp32 (1 cyc/row, ~1e-4 rel err)
#   "split" bf16 hi/lo split, 3 matmuls (3 cyc/row, ~2e-5 rel err)
MM_MODE = "f32r"

_cache = {}


def _basis_slices():
    """Per-core transposed basis slices: list of [M, R2PAD] float32 arrays."""
    if "bbT" in _cache:
        return _cache["bbT"]
    f32 = np.float32
    # phi replicated in fp32 with the reference op order
    c = np.exp(-ALPHA_X * np.linspace(0.0, MAX_TIME, NB, dtype=f32)).astype(f32)
    h = (NB / c).astype(f32)
    decay = f32(1.0 - ALPHA_X * TAU * DT)
    x = f32(1.0)
    phi = np.zeros((T - 1, NB), dtype=np.float64)
    for t in range(T - 1):
        x = f32(x * decay)
        d = (x - c).astype(f32)
        arg = (h * (d * d).astype(f32)).astype(f32)
        psi = np.exp(-arg).astype(f32)
        s = f32(psi.sum(dtype=f32))
        phi[t] = (psi.astype(np.float64) * float(x) * WEIGHT_SCALE) / float(s)

    dt = TAU * DT
    a, b = ALPHA_Y, BETA_Y
    A = np.array([[1 - dt * dt * a * b, dt * (1 - dt * a)],
                  [-dt * a * b, 1 - dt * a]], dtype=np.float64)
    B = np.array([dt * dt, dt], dtype=np.float64)
    # internal channel order: 0 homogeneous (E), 1 step (S), 2.. forced (C)
    Z = np.zeros((2, M), dtype=np.float64)
    Z[0, 0] = 1.0
    # output channel order (must match device rhs rows):
    #   m = 0..24 -> C_j (coeff w.T*(g-y0)); m = 25 -> E (coeff y0);
    #   m = 26 -> ALPHA_Y*BETA_Y*S (coeff g, scale folded into the basis)
    BB = np.zeros((T, 2, M), dtype=np.float64)
    BB[0, 0, 25] = 1.0                 # y_0 = y0 (dy_0 row stays zero)
    u = np.zeros(M)
    u[1] = 1.0
    for t in range(1, T):
        u[2:] = phi[t - 1]
        Z = A @ Z + np.outer(B, u)
        for comp in (0, 1):
            BB[t, comp, :25] = Z[comp, 2:]
            BB[t, comp, 25] = Z[comp, 0]
            BB[t, comp, 26] = (a * b) * Z[comp, 1]

    flat = np.zeros((NCORES * R2, M), dtype=f32)
    flat[: T * 2] = BB.reshape(T * 2, M).astype(f32)
    slices = []
    for i in range(NCORES):
        bbT = np.zeros((M, R2PAD), dtype=f32)
        bbT[:, :R2] = flat[i * R2:(i + 1) * R2].T
        slices.append(np.ascontiguousarray(bbT))
    _cache["bbT"] = slices
    return slices


def _program():
    """Build (once) the Bass/Tile program shared by all 8 cores."""
    if "nc" in _cache:
        return _cache["nc"]
    import concourse.mybir as mybir
    import concourse.tile as tile
    from concourse import bacc

    f32 = mybir.dt.float32
    bf16 = mybir.dt.bfloat16
    mmdt = {"f32": f32, "f32r": mybir.dt.float32r, "split": bf16}[MM_MODE]
    nc = bacc.Bacc("TRN2", target_bir_lowering=False, debug=False,
                   enable_asserts=False, num_devices=NCORES)
    bbT_h = nc.dram_tensor("bbT", [M, R2PAD], f32, kind="ExternalInput")
    y0_h = nc.dram_tensor("y0", [1, DIM], f32, kind="ExternalInput")
    g_h = nc.dram_tensor("g", [1, DIM], f32, kind="ExternalInput")
    w_h = nc.dram_tensor("w", [8, 128, NB], f32, kind="ExternalInput")
    out_h = nc.dram_tensor("out", [RPC, 3, DIM], f32, kind="ExternalOutput")

    with tile.TileContext(nc) as tc:
        with (
            tc.tile_pool(name="const", bufs=1) as const,
            tc.tile_pool(name="dram", bufs=1, space="DRAM") as dram,
            tc.tile_pool(name="psMM", bufs=6, space="PSUM") as psMM,
            tc.tile_pool(name="outp", bufs=5) as outp,
        ):
            outv = out_h.ap()

            # y0/g first: they gate the serial gmy0 -> rep_s chain that every
            # matmul depends on
            y0_s = const.tile([1, DIM], f32)
            nc.sync.dma_start(y0_s[:], y0_h.ap()[:])
            g_s = const.tile([1, DIM], f32)
            nc.sync.dma_start(g_s[:], g_h.ap()[:])
            bb_s = const.tile([M, R2PAD], f32)
            nc.sync.dma_start(bb_s[:], bbT_h.ap()[:])
            # weights tiles, free dim padded 25 -> 32 per block for the 32x32
            # DVE stream transposes (padding cols stay uninitialized: they
            # only transpose into wt rows 25..31, which are never read);
            # one strided DMA instead of 8 (each pays a ~500ns floor)
            w_s = const.tile([128, 8 * 32], f32)
            nc.sync.dma_start(
                w_s[:].rearrange("p (a j) -> p a j", a=8)[:, :, 0:NB],
                w_h.ap().rearrange("a p j -> p a j"))

            # y0-replica output block: stage y0 across 128 SBUF partitions
            # (DMA partition-broadcast needs a DRAM source), then blast it to
            # out[:, 0, :] in 128-row strided writes that read only SBUF.
            # This bulk work keeps the DMA engine busy while the matmul
            # pipeline ramps.
            rep128 = const.tile([128, DIM], f32)
            nc.sync.dma_start(rep128[:], y0_h.ap().broadcast_to([128, DIM]))
            for j in range((RPC + 127) // 128):
                rows = min(128, RPC - j * 128)
                nc.sync.dma_start(outv[j * 128:j * 128 + rows, 0, :],
                                  rep128[:rows, :])

            # g - y0, broadcast to 25 partitions via a DRAM roundtrip.
            # Issued BEFORE the bulk y0-block writes: this tiny chain gates
            # every matmul, and the DMA engine drains work in issue order.
            gmy0 = const.tile([1, DIM], f32)
            nc.vector.tensor_sub(gmy0[:], g_s[:], y0_s[:])
            gmy0_d = dram.tile([1, DIM], f32)
            nc.gpsimd.dma_start(gmy0_d[:], gmy0[:])
            rep_s = const.tile([NB, DIM], f32)
            nc.gpsimd.dma_start(rep_s[:], gmy0_d[:].broadcast_to([NB, DIM]))

            # w.T via DVE 32x32 stream transposes
            wt_s = const.tile([32, 8 * 128], f32)
            for a in range(8):
                for i in range(4):
                    nc.vector.transpose(
                        wt_s[:, a * 128 + 32 * i:a * 128 + 32 * (i + 1)],
                        w_s[32 * i:32 * (i + 1), a * 32:(a + 1) * 32])

            # rhs rows 0..24: w.T * (g - y0); rows 25/26 (y0, g) via raw DMA
            # (compute-engine APs must start at a quadrant boundary; DMA APs
            # need not)
            rhs_s = const.tile([M, DIM], f32)
            nc.vector.tensor_mul(rhs_s[0:NB, :], wt_s[0:NB, :], rep_s[:])
            nc.gpsimd.dma_start(rhs_s[NB:NB + 1, :], y0_h.ap()[:])
            nc.gpsimd.dma_start(rhs_s[NB + 1:NB + 2, :], g_h.ap()[:])
            # matmul operand precision prep.  f32/f32r: single operand pair
            # (f32r producers must round to f32r, hence the join copies).
            # split: bf16 hi/lo decomposition, out = hi*hi + hi*lo + lo*hi
            # (the dropped lo*lo term is ~2^-16 relative).
            if MM_MODE == "split":
                bbh = const.tile([M, R2PAD], bf16)
                nc.vector.tensor_copy(bbh[:], bb_s[:])
                bbl = const.tile([M, R2PAD], bf16)
                nc.vector.tensor_sub(bbl[:], bb_s[:], bbh[:])
                rhh = const.tile([M, DIM], bf16)
                nc.vector.tensor_copy(rhh[:], rhs_s[:])
                rhl = const.tile([M, DIM], bf16)
                nc.vector.tensor_sub(rhl[:], rhs_s[:], rhh[:])
            else:
                rhs2 = const.tile([M, DIM], mmdt)
                nc.vector.tensor_copy(rhs2[:], rhs_s[:])
                bb2 = const.tile([M, R2PAD], mmdt)
                nc.vector.tensor_copy(bb2[:], bb_s[:])

            # main matmul: [2502, 27] @ [27, 1024], tiled [128, 512]; each
            # 128-row tile covers 64 t-rows x {y, dy}
            for mt in range(NMT):
                ob = outp.tile([128, DIM], f32)
                ms = slice(mt * 128, (mt + 1) * 128)
                for nh in range(2):
                    ns = slice(nh * 512, (nh + 1) * 512)
                    ps = psMM.tile([128, 512], f32)
                    if MM_MODE == "split":
                        nc.tensor.matmul(ps[:], bbh[:, ms], rhl[:, ns],
                                         start=True, stop=False)
                        nc.tensor.matmul(ps[:], bbl[:, ms], rhh[:, ns],
                                         start=False, stop=False)
                        nc.tensor.matmul(ps[:], bbh[:, ms], rhh[:, ns],
                                         start=False, stop=True)
                    else:
                        nc.tensor.matmul(ps[:], bb2[:, ms], rhs2[:, ns],
                                         start=True, stop=True)
                    nc.vector.tensor_copy(ob[:, nh * 512:(nh + 1) * 512], ps[:])
                t0 = mt * 64
                tv = min(64, RPC - t0)
                nc.sync.dma_start(outv[t0:t0 + tv, 1:3, :], ob[:2 * tv, :])

    nc.compile()   # bacc passes: wait legalization (1-wait HW cap), regalloc
    _cache["nc"] = nc
    return nc


def _run(in_maps, **kwargs):
    from concourse.bass_utils import run_bass_kernel_spmd
    return run_bass_kernel_spmd(_program(), in_maps, core_ids=list(range(NCORES)),
                                **kwargs)


def _in_maps(y0, g, weights):
    f32 = np.float32
    y0f = np.ascontiguousarray(np.asarray(y0, f32).reshape(1, DIM))
    gf = np.ascontiguousarray(np.asarray(g, f32).reshape(1, DIM))
    wf = np.ascontiguousarray(np.asarray(weights, f32).reshape(8, 128, NB))
    return [{"bbT": bbT, "y0": y0f, "g": gf, "w": wf}
            for bbT in _basis_slices()]


def kernel(y0, g, weights, **_kwargs):
    res = _run(_in_maps(y0, g, weights))
    outs = [r["out"].reshape(RPC, 3 * DIM) for r in res.results]
    return np.ascontiguousarray(np.concatenate(outs, axis=0)[:T])



# revision 4
# speedup vs baseline: 1.8507x; 1.8507x over previous
"""Trainium2 Bass kernel for the DMP (dynamic movement primitives) rollout.

Math: the reference rollout is, per dimension d, a linear 2-state recurrence
    s_t = A s_{t-1} + B u_t,   s = [y; dy],  s_0 = [y0; 0]
with constant A (2x2), B = [dt^2; dt], and forcing
    u_t[d] = ALPHA_Y*BETA_Y*g[d] + sum_j phi_t[j] * weights[d,j]*(g[d]-y0[d])
where phi_t[j] = WEIGHT_SCALE * psi_t[j] * x_t / sum(psi_t) depends only on
constants (x_t = decay^t is input-independent).  By superposition the whole
trajectory factors through an input-independent basis:
    y_t[d], dy_t[d] = sum_m BB[t, comp, m] * coeff[m, d]       (m = 0..26)
with channels m = 0..24 the 25 basis-forced responses (coeff w[:,j]*(g-y0)),
m = 25 the homogeneous response (coeff y0), m = 26 the step response with
ALPHA_Y*BETA_Y folded in (coeff g).

Per core (time rows sharded across 8 cores, no cross-core comm):
  - the coeff matrix rhs[27, 1024] is built on device: per-partition scale of
    w by (g - y0) in a [128 d-part, 8 block, 32 ch] layout (y0/g ride along as
    channels 25/26, prepacked by the host), then 8 PE transposes into PSUM and
    a PSUM->SBUF copy,
  - the y/dy output blocks are a [2502, 27] @ [27, 1024] tensor-engine matmul
    in fp16 (values are O(30), fp16 rel step 2^-11 ~ 5e-4 << the 2e-2 gate),
  - outputs leave as fp16 (half the HBM write traffic of f32); the constant
    y0-replica block is assembled on the host, not written by the device.
"""

import numpy as np

DIM = 1024
NB = 25
ALPHA_X = 1.0
DT = 0.001
MAX_TIME = 10.0
TAU = 1.0
ALPHA_Y = 25.0
BETA_Y = 6.25
WEIGHT_SCALE = 1000.0
T = int(MAX_TIME / DT) + 1        # 10001

NCORES = 8
RPC = 1251                        # t-rows per core; 8*1251 = 10008 >= T
R2 = RPC * 2                      # 2502 matmul rows per core (y and dy)
R2PAD = 2560                      # 20 tiles of 128
NMT = R2PAD // 128                # 20
M = 2 + NB                        # 27 basis channels
NBLK = 8                          # 128-dim blocks of the 1024 dims
CPB = 32                          # channels per block (25 used + y0 + g + pad)

# w_ext/y0g/ident packed in one [128, IN_COLS] fp16 input tile
WE = NBLK * CPB                   # 256 w_ext cols
IN_COLS = WE + 2 * NBLK + 128     # + y0g (16) + identity (128)

_cache = {}


def _basis_slices():
    """Per-core transposed basis slices: list of [M, R2PAD] float16 arrays."""
    if "bbT" in _cache:
        return _cache["bbT"]
    f32 = np.float32
    # phi replicated in fp32 with the reference op order
    c = np.exp(-ALPHA_X * np.linspace(0.0, MAX_TIME, NB, dtype=f32)).astype(f32)
    h = (NB / c).astype(f32)
    decay = f32(1.0 - ALPHA_X * TAU * DT)
    x = f32(1.0)
    phi = np.zeros((T - 1, NB), dtype=np.float64)
    for t in range(T - 1):
        x = f32(x * decay)
        d = (x - c).astype(f32)
        arg = (h * (d * d).astype(f32)).astype(f32)
        psi = np.exp(-arg).astype(f32)
        s = f32(psi.sum(dtype=f32))
        phi[t] = (psi.astype(np.float64) * float(x) * WEIGHT_SCALE) / float(s)

    dt = TAU * DT
    a, b = ALPHA_Y, BETA_Y
    A = np.array([[1 - dt * dt * a * b, dt * (1 - dt * a)],
                  [-dt * a * b, 1 - dt * a]], dtype=np.float64)
    B = np.array([dt * dt, dt], dtype=np.float64)
    # internal channel order: 0 homogeneous (E), 1 step (S), 2.. forced (C)
    Z = np.zeros((2, M), dtype=np.float64)
    Z[0, 0] = 1.0
    # output channel order (must match device rhs rows):
    #   m = 0..24 -> C_j (coeff w.T*(g-y0)); m = 25 -> E (coeff y0);
    #   m = 26 -> ALPHA_Y*BETA_Y*S (coeff g, scale folded into the basis)
    BB = np.zeros((T, 2, M), dtype=np.float64)
    BB[0, 0, 25] = 1.0                 # y_0 = y0 (dy_0 row stays zero)
    u = np.zeros(M)
    u[1] = 1.0
    for t in range(1, T):
        u[2:] = phi[t - 1]
        Z = A @ Z + np.outer(B, u)
        for comp in (0, 1):
            BB[t, comp, :25] = Z[comp, 2:]
            BB[t, comp, 25] = Z[comp, 0]
            BB[t, comp, 26] = (a * b) * Z[comp, 1]

    flat = np.zeros((NCORES * R2, M), dtype=f32)
    flat[: T * 2] = BB.reshape(T * 2, M).astype(f32)
    slices = []
    for i in range(NCORES):
        bbT = np.zeros((M, R2PAD), dtype=np.float16)
        bbT[:, :R2] = flat[i * R2:(i + 1) * R2].T.astype(np.float16)
        slices.append(np.ascontiguousarray(bbT))
    _cache["bbT"] = slices
    return slices


def _program():
    """Build (once) the Bass/Tile program shared by all 8 cores."""
    if "nc" in _cache:
        return _cache["nc"]
    import concourse.mybir as mybir
    import concourse.tile as tile
    from concourse import bacc

    f32 = mybir.dt.float32
    f16 = mybir.dt.float16
    COPY = mybir.ActivationFunctionType.Copy
    nc = bacc.Bacc("TRN2", target_bir_lowering=False, debug=False,
                   enable_asserts=False, num_devices=NCORES)
    bbT_h = nc.dram_tensor("bbT", [M, R2PAD], f16, kind="ExternalInput")
    inb_h = nc.dram_tensor("inb", [128, IN_COLS], f16, kind="ExternalInput")
    out_h = nc.dram_tensor("out", [R2, DIM], f16, kind="ExternalOutput")

    with tile.TileContext(nc) as tc:
        with (
            tc.tile_pool(name="const", bufs=1) as const,
            tc.tile_pool(name="psT", bufs=1, space="PSUM") as psT,
            tc.tile_pool(name="psMM", bufs=3, space="PSUM") as psMM,
            tc.tile_pool(name="outp", bufs=3) as outp,
        ):
            outv = out_h.ap()

            # one fused input load: w_ext [128, 8*32] (channels 25/26 carry
            # y0/g), y0g [128, 16], identity [128, 128]; then the basis
            inb = const.tile([128, IN_COLS], f16)
            nc.sync.dma_start(inb[:], inb_h.ap()[:])
            bb2 = const.tile([M, R2PAD], f16)
            nc.sync.dma_start(bb2[:], bbT_h.ap()[:])
            w_ext = inb[:, 0:WE].rearrange("p (a j) -> p a j", a=NBLK)
            y0c = inb[:, WE:WE + NBLK]
            gc = inb[:, WE + NBLK:WE + 2 * NBLK]
            ident = inb[:, WE + 2 * NBLK:]

            # gm[p, a] = g[a*128+p] - y0[a*128+p] (f32: tensor_scalar wants
            # a float32 scalar operand)
            gm = const.tile([128, NBLK], f32)
            nc.vector.tensor_sub(gm[:], gc, y0c)

            # scale w channels 0..24 by gm, in place, per 128-dim block
            # (channels 25/26 = y0/g stay unscaled)
            for a in range(NBLK):
                nc.vector.tensor_scalar_mul(
                    w_ext[:, a, 0:NB], w_ext[:, a, 0:NB], gm[:, a:a + 1])

            # 8 PE transposes: [128 d, 32 ch] -> psum [32 ch, 128 d]
            tps = psT.tile([32, DIM], f16)
            for a in range(NBLK):
                nc.tensor.matmul(tps[:, a * 128:(a + 1) * 128],
                                 w_ext[:, a, :], ident,
                                 is_transpose=True, start=True, stop=True)

            # rhs[27, 1024] fp16: PSUM -> SBUF, halves split DVE/ACT so the
            # first main matmul can start when block a=0..3 are transposed
            rhs = const.tile([32, DIM], f16)
            nc.vector.tensor_copy(rhs[0:M, 0:512], tps[0:M, 0:512])
            nc.scalar.activation(rhs[0:M, 512:1024], tps[0:M, 512:1024], COPY)

            # main matmul: [2502, 27] @ [27, 1024] in fp16, psum tiles
            # [128, 1024] (2 banks, one matmul per 512-col bank half);
            # one PSUM->SBUF fp16 copy per tile, alternating DVE/ACT;
            # output DMAs cover 2 tiles (first/last pairs split for earlier
            # stream start / the ragged 2502-row edge)
            for mt in range(NMT):
                ms = slice(mt * 128, (mt + 1) * 128)
                if mt % 2 == 0:
                    ob = outp.tile([128, 2 * DIM], f16)
                ps = psMM.tile([128, DIM], f32)
                for nh in range(2):
                    ns = slice(nh * 512, (nh + 1) * 512)
                    nc.tensor.matmul(ps[:, ns], bb2[:, ms], rhs[0:M, ns],
                                     start=True, stop=True)
                half = slice((mt % 2) * DIM, (mt % 2 + 1) * DIM)
                if mt % 2 == 0:
                    nc.vector.tensor_copy(ob[:, half], ps[:])
                else:
                    nc.scalar.activation(ob[:, half], ps[:], COPY)

                if mt % 2 == 1:
                    r0 = (mt - 1) * 128
                    if mt == 1 or mt == NMT - 1:
                        # split pair: row counts may be ragged at the end
                        n0 = min(128, R2 - r0)
                        nc.sync.dma_start(outv[r0:r0 + n0, :],
                                          ob[0:n0, 0:DIM])
                        n1 = min(128, max(0, R2 - r0 - 128))
                        if n1 > 0:
                            nc.sync.dma_start(outv[r0 + 128:r0 + 128 + n1, :],
                                              ob[0:n1, DIM:2 * DIM])
                    else:
                        nc.sync.dma_start(
                            outv[r0:r0 + 256, :].rearrange(
                                "(h p) d -> p h d", h=2),
                            ob[:].rearrange("p (h d) -> p h d", h=2))

    nc.compile()   # bacc passes: wait legalization (1-wait HW cap), regalloc
    _cache["nc"] = nc
    return nc


def _run(in_maps, **kwargs):
    from concourse.bass_utils import run_bass_kernel_spmd
    return run_bass_kernel_spmd(_program(), in_maps, core_ids=list(range(NCORES)),
                                **kwargs)


def _in_maps(y0, g, weights):
    f16 = np.float16
    y0b = np.asarray(y0, np.float32).reshape(NBLK, 128).T   # [128, 8]
    gb = np.asarray(g, np.float32).reshape(NBLK, 128).T
    wb = np.asarray(weights, np.float32).reshape(NBLK, 128, NB)
    inb = np.zeros((128, IN_COLS), dtype=f16)
    we = inb[:, 0:WE].reshape(128, NBLK, CPB)
    we[:, :, 0:NB] = wb.transpose(1, 0, 2).astype(f16)
    we[:, :, NB] = y0b.astype(f16)
    we[:, :, NB + 1] = gb.astype(f16)
    inb[:, WE:WE + NBLK] = y0b.astype(f16)
    inb[:, WE + NBLK:WE + 2 * NBLK] = gb.astype(f16)
    inb[:, WE + 2 * NBLK:] = np.eye(128, dtype=f16)
    inb = np.ascontiguousarray(inb)
    return [{"bbT": bbT, "inb": inb} for bbT in _basis_slices()]


def kernel(y0, g, weights, **_kwargs):
    f32 = np.float32
    res = _run(_in_maps(y0, g, weights))
    out = np.empty((NCORES * RPC, 3 * DIM), dtype=f32)
    out[:, 0:DIM] = np.asarray(y0, f32).reshape(1, DIM)
    ydy = np.concatenate([r["out"].reshape(RPC, 2 * DIM) for r in res.results],
                         axis=0).astype(f32)
    out[:, DIM:] = ydy
    out[0, DIM:2 * DIM] = np.asarray(y0, f32).reshape(DIM)   # exact t=0 row
    out[0, 2 * DIM:] = 0.0
    return np.ascontiguousarray(out[:T])
